# revision 17
# baseline (speedup 1.0000x reference)
"""Distributed causal attention head on 8 TRN2 NeuronCores.

Parity-split sharding + fp8-DoubleRow projections + causal-exact
variable-width attention streams.

Sharding: core c = 2*b + p handles batch b and the KEY/VALUE blocks of
parity p (global 128-row k-blocks {2l+p}).  Each core computes partial
attention numerators Z_p^T = V_p^T P_p and partial denominators over its
k-parity for ALL q of the batch; the host sums the two partials per
batch and normalizes (flash-attention partial-softmax combine; no
max-subtraction needed since |scores/8| < ~1.5).  This removes all
output transposes/normalization from the device and halves K/V work
per core.

The Q/K projections run in fp8e4m3 with DoubleRow matmuls: X_q/X_k
arrive fp8 plane-packed by d-slice pairs, W_q/W_k arrive fp8 pre-scaled
by 32 (folded out in the exp scale), so D=512 contracts in 2 passes of
2x128 rows instead of 4 (fp8-DR doubles contraction per pass on this HW;
it does NOT stream faster, so scores/AV gain nothing from fp8 and stay
bf16 - fp8 probs/values also fail the 2e-2 accuracy gate).  Q^T/K^T are
stored bf16; PSUM is always f32.

Schedule is SPMD-identical across cores; parity lives in the DATA
(xk/xv contents and one [128,256] causal mask).  Score matmuls stream
only the causal-valid q-suffix (offset 256*l = max over parities) in
<=512-col PSUM-bank pieces; exp runs on ACT per [128,<=1024] window; the
one partially-masked 256-col region per k-block is masked on DVE after
the exp.  AV accumulates zt[65, q] in PSUM banks with start/stop per
512-col bank; finished strips are copied to SBUF and DMA'd (alternating
queues) as soon as their last k-block lands.  Projections run in a
PSUM-pool prologue (PSUM->SBUF copies alternate DVE/ACT); V is
projected bf16 and PE-transposed to k-major with a ones-column that
yields the softmax denominator for free.
"""

import numpy as np
import ml_dtypes

import concourse.bass as bass
import concourse.bacc as bacc
import concourse.mybir as mybir
import concourse.tile as tile

B, S, D, E = 4, 4096, 512, 64
PB = 128                     # partition block
NL = 16                      # local k-blocks per core (parity half)
ND = 4                       # d-slices
LAG = 2                      # ST -> AV pipeline depth (in l's)
WSC = 32.0                   # fp8 weight pre-scale (host); folded into exp
# attention q-chunks: (q0, width, Lmax)
CHUNKS = [(0, 2048, 7), (2048, 1024, 11), (3072, 1024, 15)]
BF16 = mybir.dt.bfloat16
F32 = mybir.dt.float32
F8 = mybir.dt.float8e4
NPBF16 = ml_dtypes.bfloat16
NPF8 = ml_dtypes.float8_e4m3
DR = mybir.MatmulPerfMode.DoubleRow


def st_segs(o, W):
    """512-bank-aligned segments covering [o, W)."""
    segs, a = [], o
    while a < W:
        b = min((a // 512 + 1) * 512, W)
        segs.append((a, b))
        a = b
    return segs


def l_last(q0, s, Lmax):
    """Last local k-block whose stream covers 512-col strip s of chunk."""
    return min(Lmax, (q0 + 512 * s + 511) // 256)


def build_nc():
    nc = bacc.Bacc(None)

    # fp8 Q/K inputs, plane-packed on host:
    #   xq8[p, (H,g,s,j)]: H = q-col half (2048), g = d-pair, s = d-slice in pair
    #   xk8[p, (g,s,j)]  : j over the 2048 parity-packed k cols
    #   wq8/wk8[p, (g,s,m,e)]: m = E-half (out plane), e in 0..31, pre-scaled x32
    xq8_d = nc.declare_dram_parameter("xq8", [PB, 16384], F8, isOutput=False)
    xk8_d = nc.declare_dram_parameter("xk8", [PB, 8192], F8, isOutput=False)
    wq8_d = nc.declare_dram_parameter("wq8", [PB, 256], F8, isOutput=False)
    wk8_d = nc.declare_dram_parameter("wk8", [PB, 256], F8, isOutput=False)
    xv_d = nc.declare_dram_parameter("xv", [D, S // 2], BF16, isOutput=False)
    wv_d = nc.declare_dram_parameter("wv", [D, E], BF16, isOutput=False)
    cm_d = nc.declare_dram_parameter("cmask", [PB, 256], BF16, isOutput=False)
    id_d = nc.declare_dram_parameter("ident", [E, E], BF16, isOutput=False)
    out_d = nc.declare_dram_parameter("out", [E + 1, S], F32, isOutput=True)

    with tile.TileContext(nc) as tc:
        with tc.tile_pool(name="persist", bufs=1) as pp, \
             tc.tile_pool(name="work", bufs=6) as wp, \
             tc.tile_pool(name="osb", bufs=3) as op:
            # ---- persistent SBUF tiles ----
            wq8_sb = pp.tile([PB, 256], F8, name="wq8_sb", tag="wq8_sb")
            wk8_sb = pp.tile([PB, 256], F8, name="wk8_sb", tag="wk8_sb")
            wv_sb = pp.tile([PB, ND * E], BF16, name="wv_sb", tag="wv_sb")
            mk_sb = pp.tile([PB, 256], BF16, name="mk_sb", tag="mk_sb")
            idb_sb = pp.tile([E, E], BF16, name="idb_sb", tag="idb_sb")
            xq8_sb = pp.tile([PB, 16384], F8, name="xq8_sb", tag="xq8_sb")
            xk8_sb = pp.tile([PB, 8192], F8, name="xk8_sb", tag="xk8_sb")
            xv_sb = [pp.tile([PB, 2048], BF16, name=f"xv{d}", tag=f"xv{d}")
                     for d in range(ND)]
            qpT = pp.tile([E, S], BF16, name="qpT", tag="qpT")
            kpT = pp.tile([E, S // 2], BF16, name="kpT", tag="kpT")
            vpT = pp.tile([E, S // 2], BF16, name="vpT", tag="vpT")
            vp = pp.tile([PB, NL * (E + 1)], BF16, name="vp", tag="vp")

            # plane views
            xq8v = xq8_sb[:].rearrange("p (H g s j) -> p H g s j", H=2, g=2, s=2)
            xk8v = xk8_sb[:].rearrange("p (g s j) -> p g s j", g=2, s=2)
            wq8v = wq8_sb[:].rearrange("p (g s m e) -> p g s m e", g=2, s=2, m=2)
            wk8v = wk8_sb[:].rearrange("p (g s m e) -> p g s m e", g=2, s=2, m=2)
            vpv = vp[:].rearrange("p (l e) -> p l e", e=E + 1)

            # ---- DMAs (three trigger queues: sync, gpsimd, scalar) ----
            # K-proj inputs on sync+gps, Q half 0 on the scalar queue in parallel
            nc.gpsimd.dma_start(out=wk8_sb[:], in_=wk8_d[:])
            nc.gpsimd.dma_start(out=xk8_sb[:, 4096:8192], in_=xk8_d[:, 4096:8192])
            nc.sync.dma_start(out=xk8_sb[:, 0:4096], in_=xk8_d[:, 0:4096])
            nc.scalar.dma_start(out=xq8_sb[:, 0:4096], in_=xq8_d[:, 0:4096])
            nc.scalar.dma_start(out=xq8_sb[:, 4096:8192], in_=xq8_d[:, 4096:8192])
            nc.gpsimd.dma_start(out=wq8_sb[:], in_=wq8_d[:])
            for d in range(ND):
                nc.sync.dma_start(out=xv_sb[d][:], in_=xv_d[PB * d:PB * (d + 1), :])
            nc.gpsimd.dma_start(out=xq8_sb[:, 8192:12288], in_=xq8_d[:, 8192:12288])
            nc.gpsimd.dma_start(
                out=wv_sb[:].rearrange("p (d e) -> p d e", e=E),
                in_=wv_d.rearrange("(d p) e -> p d e", p=PB))
            nc.gpsimd.dma_start(out=xq8_sb[:, 12288:16384], in_=xq8_d[:, 12288:16384])
            nc.gpsimd.dma_start(out=mk_sb[:], in_=cm_d[:])
            nc.gpsimd.dma_start(out=idb_sb[:], in_=id_d[:])

            # ones column of vp
            nc.vector.memset(vpv[:, :, E:E + 1], 1.0)

            # ---- prologue: projections ----
            copy_eng = [nc.vector, nc.scalar]
            with tc.tile_pool(name="pj8_ps", bufs=2, space="PSUM") as pj8p, \
                 tc.tile_pool(name="pjv_ps", bufs=3, space="PSUM") as pjvp, \
                 tc.tile_pool(name="vt_ps", bufs=2, space="PSUM") as vtp:
                def proj8(w8v, x8gsj, dst, dst_off, ci):
                    """One 512-col fp8 DoubleRow piece (contraction 2x256)."""
                    pj = pj8p.tile([E, 512], F32, name=f"pj8_{dst_off}_{ci}", tag="pj8")
                    for g in range(2):
                        nc.tensor.matmul(
                            pj[:],
                            w8v[:, g].rearrange("p s m e -> p s (m e)"),
                            x8gsj(g),
                            start=(g == 0), stop=(g == 1), perf_mode=DR)
                    dst_ap = dst[:, dst_off:dst_off + 512]
                    if copy_eng[ci % 2] is nc.scalar:
                        nc.scalar.copy(dst_ap, pj[:])
                    else:
                        nc.vector.tensor_copy(dst_ap, pj[:])

                ci = 0
                for i in range(4):      # K: parity half, 2048 cols
                    proj8(wk8v, lambda g, i=i: xk8v[:, g, :, 512 * i:512 * (i + 1)],
                          kpT, 512 * i, ci)
                    ci += 1
                for j in range(4):      # Q half 0
                    proj8(wq8v, lambda g, j=j: xq8v[:, 0, g, :, 512 * j:512 * (j + 1)],
                          qpT, 512 * j, ci)
                    ci += 1

                def projv(i):           # V: bf16 piece
                    pj = pjvp.tile([E, 512], F32, name=f"pjv{i}", tag="pjv")
                    for d in range(ND):
                        nc.tensor.matmul(pj[:], wv_sb[:, E * d:E * (d + 1)],
                                         xv_sb[d][:, 512 * i:512 * (i + 1)],
                                         start=(d == 0), stop=(d == ND - 1))
                    nc.vector.tensor_copy(vpT[:, 512 * i:512 * (i + 1)], pj[:])

                for i in range(4):
                    projv(i)
                # V -> k-major vp blocks (PE transpose, batched via PSUM)
                for t in range(2):
                    vt = vtp.tile([PB, 8 * E], BF16, name=f"vt{t}", tag="vt")
                    for j in range(8):
                        l = 8 * t + j
                        nc.tensor.transpose(vt[:, E * j:E * (j + 1)],
                                            vpT[:, PB * l:PB * (l + 1)],
                                            idb_sb[:])
                    nc.vector.tensor_copy(vpv[:, 8 * t:8 * t + 8, 0:E],
                                          vt[:].rearrange("p (l e) -> p l e", e=E))
                for j in range(4, 8):   # Q half 1
                    proj8(wq8v, lambda g, j=j: xq8v[:, 1, g, :, 512 * (j - 4):512 * (j - 3)],
                          qpT, 512 * j, ci)
                    ci += 1

            # ---- attention ----
            with tc.tile_pool(name="st_ps", bufs=2, space="PSUM") as stp, \
                 tc.tile_pool(name="zt_ps", bufs=1, space="PSUM") as ztp:
                for (q0, W, Lmax) in CHUNKS:
                    zt = ztp.tile([E + 1, 2048], F32, name=f"zt{q0}", tag="zt")
                    if q0 == 3072:   # last chunk: finer pieces for a short tail
                        drain_pieces = [(0, 512), (512, 768), (768, 1024)]
                    else:
                        drain_pieces = [(512 * s, 512 * (s + 1)) for s in range(W // 512)]
                    pend = []

                    def drain_av(l, o, ets, q0=q0, W=W, Lmax=Lmax, zt=zt):
                        for (a, b) in st_segs(o, W):
                            et, u = ets[a // 1024]
                            nc.tensor.matmul(
                                zt[:, a:b], vp[:, (E + 1) * l:(E + 1) * (l + 1)],
                                et[:, a - 1024 * u:b - 1024 * u],
                                start=(l == 0),
                                stop=(l == l_last(q0, a // 512, Lmax)),
                                skip_group_check=True)
                        # drain pieces whose accumulation just completed -> out
                        for (da, db) in drain_pieces:
                            ll = min(Lmax, (q0 + db - 1) // 256)
                            if ll == l:
                                zs = op.tile([E + 1, 512], F32, name=f"zs{q0}_{da}", tag="zs")
                                nc.vector.tensor_copy(zs[:, 0:db - da], zt[:, da:db])
                                oq = nc.sync if (q0 + da) % 1024 else nc.gpsimd
                                oq.dma_start(out=out_d[:, q0 + da:q0 + db],
                                             in_=zs[:, 0:db - da])

                    for l in range(Lmax + 1):
                        o = max(0, 256 * l - q0)
                        segs = st_segs(o, W)
                        st_tiles = {}
                        for u in sorted({a // 1024 for (a, _) in segs}):
                            st_tiles[u] = stp.tile([PB, 1024], F32,
                                                   name=f"st{q0}_{l}_{u}", tag="st")
                        for (a, b) in segs:
                            u = a // 1024
                            nc.tensor.matmul(
                                st_tiles[u][:, a - 1024 * u:b - 1024 * u],
                                kpT[:, PB * l:PB * (l + 1)],
                                qpT[:, q0 + a:q0 + b],
                                start=True, stop=True)
                        ets = {}
                        for u, stt in st_tiles.items():
                            et = wp.tile([PB, 1024], BF16, name=f"et{q0}_{l}_{u}", tag="et")
                            aw = max(o - 1024 * u, 0)
                            ew = min(W - 1024 * u, 1024)
                            nc.scalar.activation(
                                et[:, aw:ew], stt[:, aw:ew],
                                mybir.ActivationFunctionType.Exp,
                                scale=0.125 / (WSC * WSC))
                            ets[u] = (et, u)
                        if q0 <= 256 * l < q0 + W:   # diagonal in this chunk
                            u = o // 1024
                            mo = o - 1024 * u
                            et = ets[u][0]
                            nc.vector.tensor_mul(et[:, mo:mo + 256],
                                                 et[:, mo:mo + 256], mk_sb[:])
                        pend.append((l, o, ets))
                        if len(pend) > LAG:
                            drain_av(*pend.pop(0))
                    while pend:
                        drain_av(*pend.pop(0))
    nc.finalize()
    return nc


def make_core_inputs(key_np, value_np, query_np, Wk, Wv, Wq):
    """Host-side sharding: returns in_maps list of 8 dicts."""
    bf = lambda a: np.ascontiguousarray(a).astype(NPBF16)
    f8 = lambda a: np.ascontiguousarray(a).astype(NPF8)
    ki = np.arange(PB)[:, None]
    qi = np.arange(PB)[None, :]
    tri = (ki <= qi).astype(np.float32)
    ones = np.ones((PB, PB), np.float32)
    zeros = np.zeros((PB, PB), np.float32)

    def pack_w(Wm):  # [512, 64] -> [p, (g,s,m,e)] x WSC
        a = (Wm * WSC).reshape(2, 2, PB, 2, 32)          # (g, s, p, m, e)
        return f8(a.transpose(2, 0, 1, 3, 4).reshape(PB, 256))

    def pack_xq(Xq):  # [4096, 512] -> [p, (H,g,s,j)]
        a = Xq.T.reshape(2, 2, PB, 2, 2048)              # (g, s, p, H, j)
        return f8(a.transpose(2, 3, 0, 1, 4).reshape(PB, 16384))

    def pack_xk(XkT):  # [512, 2048] -> [p, (g,s,j)]
        a = XkT.reshape(2, 2, PB, 2048)                  # (g, s, p, j)
        return f8(a.transpose(2, 0, 1, 3).reshape(PB, 8192))

    in_maps = []
    for c in range(8):
        b, p = c // 2, c % 2
        kcols = np.concatenate(
            [np.arange(PB * (2 * l + p), PB * (2 * l + p) + PB) for l in range(NL)])
        cmask = np.concatenate([tri, ones] if p == 0 else [zeros, tri], axis=1)
        in_maps.append({
            "xq8": pack_xq(query_np[b]),
            "xk8": pack_xk(key_np[b].T[:, kcols]),
            "wq8": pack_w(Wq), "wk8": pack_w(Wk),
            "xv": bf(value_np[b].T[:, kcols]),
            "wv": bf(Wv),
            "cmask": bf(cmask),
            "ident": bf(np.eye(E, dtype=np.float32)),
        })
    return in_maps


def assemble_output(results):
    """results: 8 dicts with 'out' [65, S] f32 partials -> Z [B,S,E]."""
    Z = np.zeros((B, S, E), dtype=np.float32)
    for b in range(B):
        A = results[2 * b]["out"].astype(np.float32) + \
            results[2 * b + 1]["out"].astype(np.float32)
        Z[b] = (A[:E] / A[E:E + 1]).T
    return Z


def kernel(key_inputs, value_inputs, query_inputs, Wk, Wv, Wq):
    from concourse.bass_utils import run_bass_kernel_spmd
    nc = build_nc()
    in_maps = make_core_inputs(np.asarray(key_inputs), np.asarray(value_inputs),
                               np.asarray(query_inputs), np.asarray(Wk),
                               np.asarray(Wv), np.asarray(Wq))
    res = run_bass_kernel_spmd(nc, in_maps, core_ids=list(range(8)))
    return assemble_output(res.results)


# revision 18
# speedup vs baseline: 1.0351x; 1.0351x over previous
"""Distributed causal attention head on 8 TRN2 NeuronCores.

Parity-split sharding + fp8-DoubleRow projections + causal-exact
variable-width attention streams.

Sharding: core c = 2*b + p handles batch b and the KEY/VALUE blocks of
parity p (global 128-row k-blocks {2l+p}).  Each core computes partial
attention numerators Z_p^T = V_p^T P_p and partial denominators over its
k-parity for ALL q of the batch; the host sums the two partials per
batch and normalizes (flash-attention partial-softmax combine; no
max-subtraction needed since |scores/8| < ~1.5).  This removes all
output transposes/normalization from the device and halves K/V work
per core.

The Q/K projections run in fp8e4m3 with DoubleRow matmuls: X_q/X_k
arrive fp8 plane-packed by d-slice pairs, W_q/W_k arrive fp8 pre-scaled
by 32 (folded out in the exp scale), so D=512 contracts in 2 passes of
2x128 rows instead of 4 (fp8-DR doubles contraction per pass on this HW;
it does NOT stream faster, so scores/AV gain nothing from fp8 and stay
bf16 - fp8 probs/values also fail the 2e-2 accuracy gate).  Q^T/K^T are
stored bf16; PSUM is always f32.

Schedule is SPMD-identical across cores; parity lives in the DATA
(xk/xv contents and one [128,256] causal mask).  Score matmuls stream
only the causal-valid q-suffix (offset 256*l = max over parities) in
<=512-col PSUM-bank pieces; exp runs on ACT per [128,<=1024] window; the
one partially-masked 256-col region per k-block is masked on DVE after
the exp.  AV accumulates zt[65, q] in PSUM banks with start/stop per
512-col bank; finished strips are copied to SBUF and DMA'd (alternating
queues) as soon as their last k-block lands.  Projections run in a
PSUM-pool prologue (PSUM->SBUF copies alternate DVE/ACT); V is
projected bf16 and PE-transposed to k-major with a ones-column that
yields the softmax denominator for free.
"""

import numpy as np
import ml_dtypes

import concourse.bass as bass
import concourse.bacc as bacc
import concourse.mybir as mybir
import concourse.tile as tile

B, S, D, E = 4, 4096, 512, 64
PB = 128                     # partition block
NL = 16                      # local k-blocks per core (parity half)
ND = 4                       # d-slices
LAG = 2                      # ST -> AV pipeline depth (in l's)
WSC = 32.0                   # fp8 weight pre-scale (host); folded into exp
# attention q-chunks: (q0, width, Lmax)
CHUNKS = [(0, 2048, 7), (2048, 1024, 11), (3072, 1024, 15)]
BF16 = mybir.dt.bfloat16
F32 = mybir.dt.float32
F8 = mybir.dt.float8e4
NPBF16 = ml_dtypes.bfloat16
NPF8 = ml_dtypes.float8_e4m3
DR = mybir.MatmulPerfMode.DoubleRow


def st_segs(o, W):
    """512-bank-aligned segments covering [o, W)."""
    segs, a = [], o
    while a < W:
        b = min((a // 512 + 1) * 512, W)
        segs.append((a, b))
        a = b
    return segs


def l_last(q0, s, Lmax):
    """Last local k-block whose stream covers 512-col strip s of chunk."""
    return min(Lmax, (q0 + 512 * s + 511) // 256)


def build_nc():
    nc = bacc.Bacc(None)

    # fp8 Q/K inputs, plane-packed on host:
    #   xq8[p, (H,g,s,j)]: H = q-col half (2048), g = d-pair, s = d-slice in pair
    #   xk8[p, (g,s,j)]  : j over the 2048 parity-packed k cols
    #   wq8/wk8[p, (g,s,m,e)]: m = E-half (out plane), e in 0..31, pre-scaled x32
    xq8_d = nc.declare_dram_parameter("xq8", [PB, 16384], F8, isOutput=False)
    xk8_d = nc.declare_dram_parameter("xk8", [PB, 8192], F8, isOutput=False)
    wq8_d = nc.declare_dram_parameter("wq8", [PB, 256], F8, isOutput=False)
    wk8_d = nc.declare_dram_parameter("wk8", [PB, 256], F8, isOutput=False)
    xv_d = nc.declare_dram_parameter("xv", [D, S // 2], BF16, isOutput=False)
    wv_d = nc.declare_dram_parameter("wv", [D, E], BF16, isOutput=False)
    cm_d = nc.declare_dram_parameter("cmask", [PB, 256], BF16, isOutput=False)
    id_d = nc.declare_dram_parameter("ident", [E, E], BF16, isOutput=False)
    out_d = nc.declare_dram_parameter("out", [E + 1, S], F32, isOutput=True)

    with tile.TileContext(nc) as tc:
        with tc.tile_pool(name="persist", bufs=1) as pp, \
             tc.tile_pool(name="work", bufs=6) as wp, \
             tc.tile_pool(name="osb", bufs=3) as op:
            # ---- persistent SBUF tiles ----
            wq8_sb = pp.tile([PB, 256], F8, name="wq8_sb", tag="wq8_sb")
            wk8_sb = pp.tile([PB, 256], F8, name="wk8_sb", tag="wk8_sb")
            wv_sb = pp.tile([PB, ND * E], BF16, name="wv_sb", tag="wv_sb")
            mk_sb = pp.tile([PB, 256], BF16, name="mk_sb", tag="mk_sb")
            idb_sb = pp.tile([E, E], BF16, name="idb_sb", tag="idb_sb")
            xq8_sb = pp.tile([PB, 16384], F8, name="xq8_sb", tag="xq8_sb")
            xk8_sb = pp.tile([PB, 8192], F8, name="xk8_sb", tag="xk8_sb")
            xv_sb = [pp.tile([PB, 2048], BF16, name=f"xv{d}", tag=f"xv{d}")
                     for d in range(ND)]
            qpT = pp.tile([E, S], BF16, name="qpT", tag="qpT")
            kpT = pp.tile([E, S // 2], BF16, name="kpT", tag="kpT")
            vpT = pp.tile([E, S // 2], BF16, name="vpT", tag="vpT")
            vp = pp.tile([PB, NL * (E + 1)], BF16, name="vp", tag="vp")

            # plane views
            xq8v = xq8_sb[:].rearrange("p (H g s j) -> p H g s j", H=2, g=2, s=2)
            xk8v = xk8_sb[:].rearrange("p (g s j) -> p g s j", g=2, s=2)
            wq8v = wq8_sb[:].rearrange("p (g s m e) -> p g s m e", g=2, s=2, m=2)
            wk8v = wk8_sb[:].rearrange("p (g s m e) -> p g s m e", g=2, s=2, m=2)
            vpv = vp[:].rearrange("p (l e) -> p l e", e=E + 1)

            # ---- DMAs (two queues: sync + gpsimd) ----
            # critical path first: K-proj inputs, then Q half 0, then the rest
            nc.gpsimd.dma_start(out=wk8_sb[:], in_=wk8_d[:])
            nc.gpsimd.dma_start(out=xk8_sb[:, 4096:8192], in_=xk8_d[:, 4096:8192])
            nc.sync.dma_start(out=xk8_sb[:, 0:4096], in_=xk8_d[:, 0:4096])
            nc.gpsimd.dma_start(out=wq8_sb[:], in_=wq8_d[:])
            nc.gpsimd.dma_start(out=xq8_sb[:, 4096:8192], in_=xq8_d[:, 4096:8192])
            nc.sync.dma_start(out=xq8_sb[:, 0:4096], in_=xq8_d[:, 0:4096])
            for d in range(ND):
                nc.sync.dma_start(out=xv_sb[d][:], in_=xv_d[PB * d:PB * (d + 1), :])
            nc.gpsimd.dma_start(out=xq8_sb[:, 8192:12288], in_=xq8_d[:, 8192:12288])
            nc.gpsimd.dma_start(
                out=wv_sb[:].rearrange("p (d e) -> p d e", e=E),
                in_=wv_d.rearrange("(d p) e -> p d e", p=PB))
            nc.gpsimd.dma_start(out=xq8_sb[:, 12288:16384], in_=xq8_d[:, 12288:16384])
            nc.gpsimd.dma_start(out=mk_sb[:], in_=cm_d[:])
            nc.gpsimd.dma_start(out=idb_sb[:], in_=id_d[:])

            # ones column of vp
            nc.vector.memset(vpv[:, :, E:E + 1], 1.0)

            # ---- prologue: projections ----
            copy_eng = [nc.vector, nc.scalar]
            with tc.tile_pool(name="pj8_ps", bufs=2, space="PSUM") as pj8p, \
                 tc.tile_pool(name="pjv_ps", bufs=3, space="PSUM") as pjvp, \
                 tc.tile_pool(name="vt_ps", bufs=2, space="PSUM") as vtp:
                def proj8(w8v, x8gsj, dst, dst_off, ci):
                    """One 512-col fp8 DoubleRow piece (contraction 2x256)."""
                    pj = pj8p.tile([E, 512], F32, name=f"pj8_{dst_off}_{ci}", tag="pj8")
                    for g in range(2):
                        nc.tensor.matmul(
                            pj[:],
                            w8v[:, g].rearrange("p s m e -> p s (m e)"),
                            x8gsj(g),
                            start=(g == 0), stop=(g == 1), perf_mode=DR)
                    dst_ap = dst[:, dst_off:dst_off + 512]
                    if copy_eng[ci % 2] is nc.scalar:
                        nc.scalar.copy(dst_ap, pj[:])
                    else:
                        nc.vector.tensor_copy(dst_ap, pj[:])

                ci = 0
                for i in range(4):      # K: parity half, 2048 cols
                    proj8(wk8v, lambda g, i=i: xk8v[:, g, :, 512 * i:512 * (i + 1)],
                          kpT, 512 * i, ci)
                    ci += 1
                for j in range(4):      # Q half 0
                    proj8(wq8v, lambda g, j=j: xq8v[:, 0, g, :, 512 * j:512 * (j + 1)],
                          qpT, 512 * j, ci)
                    ci += 1

                def projv(i):           # V: bf16 piece
                    pj = pjvp.tile([E, 512], F32, name=f"pjv{i}", tag="pjv")
                    for d in range(ND):
                        nc.tensor.matmul(pj[:], wv_sb[:, E * d:E * (d + 1)],
                                         xv_sb[d][:, 512 * i:512 * (i + 1)],
                                         start=(d == 0), stop=(d == ND - 1))
                    nc.vector.tensor_copy(vpT[:, 512 * i:512 * (i + 1)], pj[:])

                for i in range(4):
                    projv(i)
                # V -> k-major vp blocks (PE transpose, batched via PSUM)
                for t in range(2):
                    vt = vtp.tile([PB, 8 * E], BF16, name=f"vt{t}", tag="vt")
                    for j in range(8):
                        l = 8 * t + j
                        nc.tensor.transpose(vt[:, E * j:E * (j + 1)],
                                            vpT[:, PB * l:PB * (l + 1)],
                                            idb_sb[:])
                    nc.vector.tensor_copy(vpv[:, 8 * t:8 * t + 8, 0:E],
                                          vt[:].rearrange("p (l e) -> p l e", e=E))
                for j in range(4, 8):   # Q half 1
                    proj8(wq8v, lambda g, j=j: xq8v[:, 1, g, :, 512 * (j - 4):512 * (j - 3)],
                          qpT, 512 * j, ci)
                    ci += 1

            # ---- attention ----
            with tc.tile_pool(name="st_ps", bufs=2, space="PSUM") as stp, \
                 tc.tile_pool(name="zt_ps", bufs=1, space="PSUM") as ztp:
                for (q0, W, Lmax) in CHUNKS:
                    zt = ztp.tile([E + 1, 2048], F32, name=f"zt{q0}", tag="zt")
                    if q0 == 3072:   # last chunk: finer pieces for a short tail
                        drain_pieces = [(0, 512), (512, 768), (768, 1024)]
                    else:
                        drain_pieces = [(512 * s, 512 * (s + 1)) for s in range(W // 512)]
                    pend = []

                    def drain_av(l, o, ets, q0=q0, W=W, Lmax=Lmax, zt=zt):
                        for (a, b) in st_segs(o, W):
                            et, u = ets[a // 1024]
                            nc.tensor.matmul(
                                zt[:, a:b], vp[:, (E + 1) * l:(E + 1) * (l + 1)],
                                et[:, a - 1024 * u:b - 1024 * u],
                                start=(l == 0),
                                stop=(l == l_last(q0, a // 512, Lmax)),
                                skip_group_check=True)
                        # drain pieces whose accumulation just completed -> out
                        for (da, db) in drain_pieces:
                            ll = min(Lmax, (q0 + db - 1) // 256)
                            if ll == l:
                                zs = op.tile([E + 1, 512], F32, name=f"zs{q0}_{da}", tag="zs")
                                nc.vector.tensor_copy(zs[:, 0:db - da], zt[:, da:db])
                                oq = nc.sync if (q0 + da) % 1024 else nc.gpsimd
                                oq.dma_start(out=out_d[:, q0 + da:q0 + db],
                                             in_=zs[:, 0:db - da])

                    for l in range(Lmax + 1):
                        o = max(0, 256 * l - q0)
                        segs = st_segs(o, W)
                        st_tiles = {}
                        for u in sorted({a // 1024 for (a, _) in segs}):
                            st_tiles[u] = stp.tile([PB, 1024], F32,
                                                   name=f"st{q0}_{l}_{u}", tag="st")
                        for (a, b) in segs:
                            u = a // 1024
                            nc.tensor.matmul(
                                st_tiles[u][:, a - 1024 * u:b - 1024 * u],
                                kpT[:, PB * l:PB * (l + 1)],
                                qpT[:, q0 + a:q0 + b],
                                start=True, stop=True)
                        ets = {}
                        for u, stt in st_tiles.items():
                            et = wp.tile([PB, 1024], BF16, name=f"et{q0}_{l}_{u}", tag="et")
                            aw = max(o - 1024 * u, 0)
                            ew = min(W - 1024 * u, 1024)
                            nc.scalar.activation(
                                et[:, aw:ew], stt[:, aw:ew],
                                mybir.ActivationFunctionType.Exp,
                                scale=0.125 / (WSC * WSC))
                            ets[u] = (et, u)
                        if q0 <= 256 * l < q0 + W:   # diagonal in this chunk
                            u = o // 1024
                            mo = o - 1024 * u
                            et = ets[u][0]
                            nc.vector.tensor_mul(et[:, mo:mo + 256],
                                                 et[:, mo:mo + 256], mk_sb[:])
                        pend.append((l, o, ets))
                        if len(pend) > LAG:
                            drain_av(*pend.pop(0))
                    while pend:
                        drain_av(*pend.pop(0))
    nc.finalize()
    return nc


def make_core_inputs(key_np, value_np, query_np, Wk, Wv, Wq):
    """Host-side sharding: returns in_maps list of 8 dicts."""
    bf = lambda a: np.ascontiguousarray(a).astype(NPBF16)
    f8 = lambda a: np.ascontiguousarray(a).astype(NPF8)
    ki = np.arange(PB)[:, None]
    qi = np.arange(PB)[None, :]
    tri = (ki <= qi).astype(np.float32)
    ones = np.ones((PB, PB), np.float32)
    zeros = np.zeros((PB, PB), np.float32)

    def pack_w(Wm):  # [512, 64] -> [p, (g,s,m,e)] x WSC
        a = (Wm * WSC).reshape(2, 2, PB, 2, 32)          # (g, s, p, m, e)
        return f8(a.transpose(2, 0, 1, 3, 4).reshape(PB, 256))

    def pack_xq(Xq):  # [4096, 512] -> [p, (H,g,s,j)]
        a = Xq.T.reshape(2, 2, PB, 2, 2048)              # (g, s, p, H, j)
        return f8(a.transpose(2, 3, 0, 1, 4).reshape(PB, 16384))

    def pack_xk(XkT):  # [512, 2048] -> [p, (g,s,j)]
        a = XkT.reshape(2, 2, PB, 2048)                  # (g, s, p, j)
        return f8(a.transpose(2, 0, 1, 3).reshape(PB, 8192))

    in_maps = []
    for c in range(8):
        b, p = c // 2, c % 2
        kcols = np.concatenate(
            [np.arange(PB * (2 * l + p), PB * (2 * l + p) + PB) for l in range(NL)])
        cmask = np.concatenate([tri, ones] if p == 0 else [zeros, tri], axis=1)
        in_maps.append({
            "xq8": pack_xq(query_np[b]),
            "xk8": pack_xk(key_np[b].T[:, kcols]),
            "wq8": pack_w(Wq), "wk8": pack_w(Wk),
            "xv": bf(value_np[b].T[:, kcols]),
            "wv": bf(Wv),
            "cmask": bf(cmask),
            "ident": bf(np.eye(E, dtype=np.float32)),
        })
    return in_maps


def assemble_output(results):
    """results: 8 dicts with 'out' [65, S] f32 partials -> Z [B,S,E]."""
    Z = np.zeros((B, S, E), dtype=np.float32)
    for b in range(B):
        A = results[2 * b]["out"].astype(np.float32) + \
            results[2 * b + 1]["out"].astype(np.float32)
        Z[b] = (A[:E] / A[E:E + 1]).T
    return Z


def kernel(key_inputs, value_inputs, query_inputs, Wk, Wv, Wq):
    from concourse.bass_utils import run_bass_kernel_spmd
    nc = build_nc()
    in_maps = make_core_inputs(np.asarray(key_inputs), np.asarray(value_inputs),
                               np.asarray(query_inputs), np.asarray(Wk),
                               np.asarray(Wv), np.asarray(Wq))
    res = run_bass_kernel_spmd(nc, in_maps, core_ids=list(range(8)))
    return assemble_output(res.results)


# revision 19
# speedup vs baseline: 1.2035x; 1.1627x over previous
"""Distributed causal attention head on 8 TRN2 NeuronCores.

Parity-split sharding + fp8-DoubleRow projections + causal-exact
variable-width attention streams.

Sharding: core c = 2*b + p handles batch b and the KEY/VALUE blocks of
parity p (global 128-row k-blocks {2l+p}).  Each core computes partial
attention numerators Z_p^T = V_p^T P_p and partial denominators over its
k-parity for ALL q of the batch; the host sums the two partials per
batch and normalizes (flash-attention partial-softmax combine; no
max-subtraction needed since |scores/8| < ~1.5).  This removes all
output transposes/normalization from the device and halves K/V work
per core.

The Q/K projections run in fp8e4m3 with DoubleRow matmuls: X_q/X_k
arrive fp8 plane-packed by d-slice pairs, W_q/W_k arrive fp8 pre-scaled
by 32 (folded out in the exp scale), so D=512 contracts in 2 passes of
2x128 rows instead of 4 (fp8-DR doubles contraction per pass on this HW;
it does NOT stream faster, so scores/AV gain nothing from fp8 and stay
bf16 - fp8 probs/values also fail the 2e-2 accuracy gate).  Q^T/K^T are
stored bf16; PSUM is always f32.

Schedule is SPMD-identical across cores; parity lives in the DATA
(xk/xv contents and one [128,256] causal mask).  Score matmuls stream
only the causal-valid q-suffix (offset 256*l = max over parities) in
<=512-col PSUM-bank pieces; exp runs on ACT per [128,<=1024] window; the
one partially-masked 256-col region per k-block is masked on DVE after
the exp.  AV accumulates zt[65, q] in PSUM banks with start/stop per
512-col bank; finished strips are copied to SBUF and DMA'd (alternating
queues) as soon as their last k-block lands.  Projections run in a
PSUM-pool prologue (PSUM->SBUF copies alternate DVE/ACT); V is
projected bf16 and PE-transposed to k-major with a ones-column that
yields the softmax denominator for free.
"""

import numpy as np
import ml_dtypes

import concourse.bass as bass
import concourse.bacc as bacc
import concourse.mybir as mybir
import concourse.tile as tile

B, S, D, E = 4, 4096, 512, 64
PB = 128                     # partition block
NL = 16                      # local k-blocks per core (parity half)
ND = 4                       # d-slices
LAG = 3                      # ST -> AV pipeline depth (in l's)
WSC = 32.0                   # fp8 weight pre-scale (host); folded into exp
# attention q-chunks: (q0, width, Lmax)
CHUNKS = [(0, 2048, 7), (2048, 1024, 11), (3072, 1024, 15)]
BF16 = mybir.dt.bfloat16
F32 = mybir.dt.float32
F8 = mybir.dt.float8e4
NPBF16 = ml_dtypes.bfloat16
NPF8 = ml_dtypes.float8_e4m3
DR = mybir.MatmulPerfMode.DoubleRow


def st_segs(o, W):
    """512-bank-aligned segments covering [o, W)."""
    segs, a = [], o
    while a < W:
        b = min((a // 512 + 1) * 512, W)
        segs.append((a, b))
        a = b
    return segs


def l_last(q0, s, Lmax):
    """Last local k-block whose stream covers 512-col strip s of chunk."""
    return min(Lmax, (q0 + 512 * s + 511) // 256)


def build_nc():
    nc = bacc.Bacc(None)

    # fp8 Q/K inputs, plane-packed on host:
    #   xq8[p, (H,g,s,j)]: H = q-col half (2048), g = d-pair, s = d-slice in pair
    #   xk8[p, (g,s,j)]  : j over the 2048 parity-packed k cols
    #   wq8/wk8[p, (g,s,m,e)]: m = E-half (out plane), e in 0..31, pre-scaled x32
    xq8_d = nc.declare_dram_parameter("xq8", [PB, 16384], F8, isOutput=False)
    xk8_d = nc.declare_dram_parameter("xk8", [PB, 8192], F8, isOutput=False)
    wq8_d = nc.declare_dram_parameter("wq8", [PB, 256], F8, isOutput=False)
    wk8_d = nc.declare_dram_parameter("wk8", [PB, 256], F8, isOutput=False)
    xv_d = nc.declare_dram_parameter("xv", [D, S // 2], BF16, isOutput=False)
    wv_d = nc.declare_dram_parameter("wv", [D, E], BF16, isOutput=False)
    cm_d = nc.declare_dram_parameter("cmask", [PB, 256], BF16, isOutput=False)
    id_d = nc.declare_dram_parameter("ident", [E, E], BF16, isOutput=False)
    out_d = nc.declare_dram_parameter("out", [E + 1, S], F32, isOutput=True)

    with tile.TileContext(nc) as tc:
        with tc.tile_pool(name="persist", bufs=1) as pp, \
             tc.tile_pool(name="work", bufs=8) as wp, \
             tc.tile_pool(name="osb", bufs=3) as op:
            # ---- persistent SBUF tiles ----
            wq8_sb = pp.tile([PB, 256], F8, name="wq8_sb", tag="wq8_sb")
            wk8_sb = pp.tile([PB, 256], F8, name="wk8_sb", tag="wk8_sb")
            wv_sb = pp.tile([PB, ND * E], BF16, name="wv_sb", tag="wv_sb")
            mk_sb = pp.tile([PB, 256], BF16, name="mk_sb", tag="mk_sb")
            idb_sb = pp.tile([E, E], BF16, name="idb_sb", tag="idb_sb")
            xq8_sb = pp.tile([PB, 16384], F8, name="xq8_sb", tag="xq8_sb")
            xk8_sb = pp.tile([PB, 8192], F8, name="xk8_sb", tag="xk8_sb")
            xv_sb = [pp.tile([PB, 2048], BF16, name=f"xv{d}", tag=f"xv{d}")
                     for d in range(ND)]
            qpT = pp.tile([E, S], BF16, name="qpT", tag="qpT")
            kpT = pp.tile([E, S // 2], BF16, name="kpT", tag="kpT")
            vpT = pp.tile([E, S // 2], BF16, name="vpT", tag="vpT")
            vp = pp.tile([PB, NL * (E + 1)], BF16, name="vp", tag="vp")

            # plane views
            xq8v = xq8_sb[:].rearrange("p (H g s j) -> p H g s j", H=2, g=2, s=2)
            xk8v = xk8_sb[:].rearrange("p (g s j) -> p g s j", g=2, s=2)
            wq8v = wq8_sb[:].rearrange("p (g s m e) -> p g s m e", g=2, s=2, m=2)
            wk8v = wk8_sb[:].rearrange("p (g s m e) -> p g s m e", g=2, s=2, m=2)
            vpv = vp[:].rearrange("p (l e) -> p l e", e=E + 1)

            # ---- DMAs (two queues: sync + gpsimd) ----
            # critical path first: K-proj inputs, then Q half 0, then the rest
            nc.gpsimd.dma_start(out=wk8_sb[:], in_=wk8_d[:])
            nc.gpsimd.dma_start(out=xk8_sb[:, 4096:8192], in_=xk8_d[:, 4096:8192])
            nc.sync.dma_start(out=xk8_sb[:, 0:4096], in_=xk8_d[:, 0:4096])
            nc.gpsimd.dma_start(out=wq8_sb[:], in_=wq8_d[:])
            nc.gpsimd.dma_start(out=xq8_sb[:, 4096:8192], in_=xq8_d[:, 4096:8192])
            nc.sync.dma_start(out=xq8_sb[:, 0:4096], in_=xq8_d[:, 0:4096])
            for d in range(ND):
                nc.sync.dma_start(out=xv_sb[d][:], in_=xv_d[PB * d:PB * (d + 1), :])
            nc.gpsimd.dma_start(out=xq8_sb[:, 8192:12288], in_=xq8_d[:, 8192:12288])
            nc.gpsimd.dma_start(
                out=wv_sb[:].rearrange("p (d e) -> p d e", e=E),
                in_=wv_d.rearrange("(d p) e -> p d e", p=PB))
            nc.gpsimd.dma_start(out=xq8_sb[:, 12288:16384], in_=xq8_d[:, 12288:16384])
            nc.gpsimd.dma_start(out=mk_sb[:], in_=cm_d[:])
            nc.gpsimd.dma_start(out=idb_sb[:], in_=id_d[:])

            # ones column of vp
            nc.vector.memset(vpv[:, :, E:E + 1], 1.0)

            # ---- prologue: projections ----
            copy_eng = [nc.vector, nc.scalar]
            with tc.tile_pool(name="pj8_ps", bufs=2, space="PSUM") as pj8p, \
                 tc.tile_pool(name="pjv_ps", bufs=3, space="PSUM") as pjvp, \
                 tc.tile_pool(name="vt_ps", bufs=2, space="PSUM") as vtp:
                def proj8(w8v, x8gsj, dst, dst_off, ci):
                    """One 512-col fp8 DoubleRow piece (contraction 2x256)."""
                    pj = pj8p.tile([E, 512], F32, name=f"pj8_{dst_off}_{ci}", tag="pj8")
                    for g in range(2):
                        nc.tensor.matmul(
                            pj[:],
                            w8v[:, g].rearrange("p s m e -> p s (m e)"),
                            x8gsj(g),
                            start=(g == 0), stop=(g == 1), perf_mode=DR)
                    dst_ap = dst[:, dst_off:dst_off + 512]
                    if copy_eng[ci % 2] is nc.scalar:
                        nc.scalar.copy(dst_ap, pj[:])
                    else:
                        nc.vector.tensor_copy(dst_ap, pj[:])

                ci = 0
                for i in range(4):      # K: parity half, 2048 cols
                    proj8(wk8v, lambda g, i=i: xk8v[:, g, :, 512 * i:512 * (i + 1)],
                          kpT, 512 * i, ci)
                    ci += 1
                for j in range(4):      # Q half 0
                    proj8(wq8v, lambda g, j=j: xq8v[:, 0, g, :, 512 * j:512 * (j + 1)],
                          qpT, 512 * j, ci)
                    ci += 1

                def projv(i):           # V: bf16 piece
                    pj = pjvp.tile([E, 512], F32, name=f"pjv{i}", tag="pjv")
                    for d in range(ND):
                        nc.tensor.matmul(pj[:], wv_sb[:, E * d:E * (d + 1)],
                                         xv_sb[d][:, 512 * i:512 * (i + 1)],
                                         start=(d == 0), stop=(d == ND - 1))
                    nc.vector.tensor_copy(vpT[:, 512 * i:512 * (i + 1)], pj[:])

                for i in range(4):
                    projv(i)
                # V -> k-major vp blocks (PE transpose, batched via PSUM)
                for t in range(2):
                    vt = vtp.tile([PB, 8 * E], BF16, name=f"vt{t}", tag="vt")
                    for j in range(8):
                        l = 8 * t + j
                        nc.tensor.transpose(vt[:, E * j:E * (j + 1)],
                                            vpT[:, PB * l:PB * (l + 1)],
                                            idb_sb[:])
                    nc.vector.tensor_copy(vpv[:, 8 * t:8 * t + 8, 0:E],
                                          vt[:].rearrange("p (l e) -> p l e", e=E))
                for j in range(4, 8):   # Q half 1
                    proj8(wq8v, lambda g, j=j: xq8v[:, 1, g, :, 512 * (j - 4):512 * (j - 3)],
                          qpT, 512 * j, ci)
                    ci += 1

            # ---- attention ----
            with tc.tile_pool(name="st_ps", bufs=2, space="PSUM") as stp, \
                 tc.tile_pool(name="zt_ps", bufs=1, space="PSUM") as ztp:
                for (q0, W, Lmax) in CHUNKS:
                    zt = ztp.tile([E + 1, 2048], F32, name=f"zt{q0}", tag="zt")
                    if q0 == 3072:   # last chunk: finer pieces for a short tail
                        drain_pieces = [(0, 512), (512, 768), (768, 1024)]
                    else:
                        drain_pieces = [(512 * s, 512 * (s + 1)) for s in range(W // 512)]
                    pend = []

                    def drain_av(l, o, ets, q0=q0, W=W, Lmax=Lmax, zt=zt):
                        for (a, b) in st_segs(o, W):
                            et, u = ets[a // 1024]
                            nc.tensor.matmul(
                                zt[:, a:b], vp[:, (E + 1) * l:(E + 1) * (l + 1)],
                                et[:, a - 1024 * u:b - 1024 * u],
                                start=(l == 0),
                                stop=(l == l_last(q0, a // 512, Lmax)),
                                skip_group_check=True)
                        # drain pieces whose accumulation just completed -> out
                        for (da, db) in drain_pieces:
                            ll = min(Lmax, (q0 + db - 1) // 256)
                            if ll == l:
                                zs = op.tile([E + 1, 512], F32, name=f"zs{q0}_{da}", tag="zs")
                                nc.vector.tensor_copy(zs[:, 0:db - da], zt[:, da:db])
                                oq = nc.sync if (q0 + da) % 1024 else nc.gpsimd
                                oq.dma_start(out=out_d[:, q0 + da:q0 + db],
                                             in_=zs[:, 0:db - da])

                    for l in range(Lmax + 1):
                        o = max(0, 256 * l - q0)
                        segs = st_segs(o, W)
                        st_tiles = {}
                        for u in sorted({a // 1024 for (a, _) in segs}):
                            st_tiles[u] = stp.tile([PB, 1024], F32,
                                                   name=f"st{q0}_{l}_{u}", tag="st")
                        for (a, b) in segs:
                            u = a // 1024
                            nc.tensor.matmul(
                                st_tiles[u][:, a - 1024 * u:b - 1024 * u],
                                kpT[:, PB * l:PB * (l + 1)],
                                qpT[:, q0 + a:q0 + b],
                                start=True, stop=True)
                        ets = {}
                        for u, stt in st_tiles.items():
                            et = wp.tile([PB, 1024], BF16, name=f"et{q0}_{l}_{u}", tag="et")
                            aw = max(o - 1024 * u, 0)
                            ew = min(W - 1024 * u, 1024)
                            nc.scalar.activation(
                                et[:, aw:ew], stt[:, aw:ew],
                                mybir.ActivationFunctionType.Exp,
                                scale=0.125 / (WSC * WSC))
                            ets[u] = (et, u)
                        if q0 <= 256 * l < q0 + W:   # diagonal in this chunk
                            u = o // 1024
                            mo = o - 1024 * u
                            et = ets[u][0]
                            nc.vector.tensor_mul(et[:, mo:mo + 256],
                                                 et[:, mo:mo + 256], mk_sb[:])
                        pend.append((l, o, ets))
                        if len(pend) > LAG:
                            drain_av(*pend.pop(0))
                    while pend:
                        drain_av(*pend.pop(0))
    nc.finalize()
    return nc


def make_core_inputs(key_np, value_np, query_np, Wk, Wv, Wq):
    """Host-side sharding: returns in_maps list of 8 dicts."""
    bf = lambda a: np.ascontiguousarray(a).astype(NPBF16)
    f8 = lambda a: np.ascontiguousarray(a).astype(NPF8)
    ki = np.arange(PB)[:, None]
    qi = np.arange(PB)[None, :]
    tri = (ki <= qi).astype(np.float32)
    ones = np.ones((PB, PB), np.float32)
    zeros = np.zeros((PB, PB), np.float32)

    def pack_w(Wm):  # [512, 64] -> [p, (g,s,m,e)] x WSC
        a = (Wm * WSC).reshape(2, 2, PB, 2, 32)          # (g, s, p, m, e)
        return f8(a.transpose(2, 0, 1, 3, 4).reshape(PB, 256))

    def pack_xq(Xq):  # [4096, 512] -> [p, (H,g,s,j)]
        a = Xq.T.reshape(2, 2, PB, 2, 2048)              # (g, s, p, H, j)
        return f8(a.transpose(2, 3, 0, 1, 4).reshape(PB, 16384))

    def pack_xk(XkT):  # [512, 2048] -> [p, (g,s,j)]
        a = XkT.reshape(2, 2, PB, 2048)                  # (g, s, p, j)
        return f8(a.transpose(2, 0, 1, 3).reshape(PB, 8192))

    in_maps = []
    for c in range(8):
        b, p = c // 2, c % 2
        kcols = np.concatenate(
            [np.arange(PB * (2 * l + p), PB * (2 * l + p) + PB) for l in range(NL)])
        cmask = np.concatenate([tri, ones] if p == 0 else [zeros, tri], axis=1)
        in_maps.append({
            "xq8": pack_xq(query_np[b]),
            "xk8": pack_xk(key_np[b].T[:, kcols]),
            "wq8": pack_w(Wq), "wk8": pack_w(Wk),
            "xv": bf(value_np[b].T[:, kcols]),
            "wv": bf(Wv),
            "cmask": bf(cmask),
            "ident": bf(np.eye(E, dtype=np.float32)),
        })
    return in_maps


def assemble_output(results):
    """results: 8 dicts with 'out' [65, S] f32 partials -> Z [B,S,E]."""
    Z = np.zeros((B, S, E), dtype=np.float32)
    for b in range(B):
        A = results[2 * b]["out"].astype(np.float32) + \
            results[2 * b + 1]["out"].astype(np.float32)
        Z[b] = (A[:E] / A[E:E + 1]).T
    return Z


def kernel(key_inputs, value_inputs, query_inputs, Wk, Wv, Wq):
    from concourse.bass_utils import run_bass_kernel_spmd
    nc = build_nc()
    in_maps = make_core_inputs(np.asarray(key_inputs), np.asarray(value_inputs),
                               np.asarray(query_inputs), np.asarray(Wk),
                               np.asarray(Wv), np.asarray(Wq))
    res = run_bass_kernel_spmd(nc, in_maps, core_ids=list(range(8)))
    return assemble_output(res.results)
